# revision 1
# baseline (speedup 1.0000x reference)
"""Gaussian falloff vortex-velocity kernel for Trainium2 (8 NeuronCores).

Math: out[b,h,w,:] = sum_n tau_n * exp(-r2/sig_n^2) / sqrt(r2) * (d2, -d1)
with d1 = py - y_n, d2 = px - x_n, r2 = d1^2 + d2^2.

Key identities used on device:
  exp(-r2/sig^2)/sqrt(r2) = exp(nisg*(r2 + c*ln(r2))),  nisg=-1/sig^2, c=0.5*sig^2
  out_u = px*S0 - S1,  out_v = S2 - py*S0
  where S0 = sum tau*g, S1 = sum tau*x*g, S2 = sum tau*y*g  (3-col matmul over n)

Sharding: grid rows H are split across the 8 cores (32 rows each); every core
streams all 512 particles (replicated params).
"""

import sys

import numpy as np

B, H, W, N = 2, 256, 256, 512
NCORES = 8
HPC = H // NCORES          # 32 rows per core
PPB = HPC * W              # 8192 points per batch per core
NT = PPB // 512            # 16 point-tiles of 512 per batch
NK = N // 128              # 4 particle blocks

_cache = {}


def _bass_modules():
    if "/opt/trn_rl_repo" not in sys.path:
        sys.path.insert(0, "/opt/trn_rl_repo")
    import concourse.bass as bass
    import concourse.mybir as mybir
    import concourse.tile as tile
    from concourse import bacc
    from concourse.bass_utils import run_bass_kernel_spmd

    return bass, mybir, tile, run_bass_kernel_spmd, bacc


def _build_nc():
    bass, mybir, tile, _, bacc = _bass_modules()
    f32 = mybir.dt.float32
    AF = mybir.ActivationFunctionType
    ALU = mybir.AluOpType

    nc = bacc.Bacc(None)
    params_d = nc.declare_dram_parameter("params", [128, 32], f32, isOutput=False)
    wmat_d = nc.declare_dram_parameter("wmat", [128, 24], f32, isOutput=False)
    ptsb_d = nc.declare_dram_parameter("ptsb", [B, NT, 2, 128, 512], f32, isOutput=False)
    ptsf_d = nc.declare_dram_parameter("ptsf", [B, 2, 128, PPB // 128], f32, isOutput=False)
    out_d = nc.declare_dram_parameter("out", [B, 2, 128, PPB // 128], f32, isOutput=True)

    with tile.TileContext(nc) as tc:
        with (
            tc.tile_pool(name="const", bufs=1) as cpool,
            tc.tile_pool(name="temps", bufs=3) as temps,
            tc.tile_pool(name="psbc", bufs=2) as psbc,
            tc.tile_pool(name="psacc", bufs=2, space=bass.MemorySpace.PSUM) as psacc,
            tc.tile_pool(name="fin", bufs=2) as fin,
            tc.tile_pool(name="dscratch", bufs=1, space="DRAM") as dpool,
        ):
            params = cpool.tile([128, 32], f32)
            nc.sync.dma_start(params[:], params_d[:])
            wmat = cpool.tile([128, 24], f32)
            nc.sync.dma_start(wmat[:], wmat_d[:])
            scratch = dpool.tile([B, 3, PPB], f32)

            for b in range(B):
                for T in range(NT):
                    sl = slice(T * 512, (T + 1) * 512)
                    pyb = psbc.tile([128, 512], f32, tag="pyb")
                    nc.sync.dma_start(pyb[:], ptsb_d[b, T, 0])
                    pxb = psbc.tile([128, 512], f32, tag="pxb")
                    nc.sync.dma_start(pxb[:], ptsb_d[b, T, 1])
                    sacc = psacc.tile([3, 512], f32, tag="sacc")
                    for k in range(NK):
                        c = b * NK + k
                        d1sq = temps.tile([128, 512], f32, tag="d1sq")
                        nc.scalar.activation(
                            d1sq[:], pyb[:], AF.Square, bias=params[:, c : c + 1]
                        )
                        d2 = temps.tile([128, 512], f32, tag="d2")
                        nc.vector.tensor_scalar_add(
                            d2[:], pxb[:], params[:, 8 + c : 9 + c]
                        )
                        d2sq = temps.tile([128, 512], f32, tag="d2sq")
                        nc.gpsimd.tensor_mul(d2sq[:], d2[:], d2[:])
                        r2 = temps.tile([128, 512], f32, tag="r2")
                        nc.vector.tensor_add(r2[:], d1sq[:], d2sq[:])
                        lt = temps.tile([128, 512], f32, tag="lt")
                        nc.scalar.activation(lt[:], r2[:], AF.Ln)
                        wt = temps.tile([128, 512], f32, tag="wt")
                        nc.vector.scalar_tensor_tensor(
                            wt[:], lt[:], params[:, 24 + c : 25 + c], r2[:],
                            ALU.mult, ALU.add,
                        )
                        g = temps.tile([128, 512], f32, tag="g")
                        nc.scalar.activation(
                            g[:], wt[:], AF.Exp, scale=params[:, 16 + c : 17 + c]
                        )
                        nc.tensor.matmul(
                            sacc[:], wmat[:, c * 3 : (c + 1) * 3], g[:],
                            start=(k == 0), stop=(k == NK - 1),
                        )
                    srow = temps.tile([3, 512], f32, tag="srow")
                    nc.scalar.copy(srow[:], sacc[:])
                    nc.sync.dma_start(scratch[b, :, sl], srow[:])

            srs = scratch[:].rearrange("b three (p f) -> b three p f", p=128)
            for b in range(B):
                s0 = fin.tile([128, PPB // 128], f32, tag="s0")
                nc.sync.dma_start(s0[:], srs[b, 0])
                s1 = fin.tile([128, PPB // 128], f32, tag="s1")
                nc.sync.dma_start(s1[:], srs[b, 1])
                s2 = fin.tile([128, PPB // 128], f32, tag="s2")
                nc.sync.dma_start(s2[:], srs[b, 2])
                pyf = fin.tile([128, PPB // 128], f32, tag="pyf")
                nc.sync.dma_start(pyf[:], ptsf_d[b, 0])
                pxf = fin.tile([128, PPB // 128], f32, tag="pxf")
                nc.sync.dma_start(pxf[:], ptsf_d[b, 1])
                tu = fin.tile([128, PPB // 128], f32, tag="tu")
                nc.vector.tensor_mul(tu[:], pxf[:], s0[:])
                u = fin.tile([128, PPB // 128], f32, tag="u")
                nc.vector.tensor_sub(u[:], tu[:], s1[:])
                tv = fin.tile([128, PPB // 128], f32, tag="tv")
                nc.vector.tensor_mul(tv[:], pyf[:], s0[:])
                v = fin.tile([128, PPB // 128], f32, tag="v")
                nc.vector.tensor_sub(v[:], s2[:], tv[:])
                nc.sync.dma_start(out_d[b, 0], u[:])
                nc.sync.dma_start(out_d[b, 1], v[:])
    nc.compile()
    return nc


def _prep_inputs(vortex_feature, points):
    vf = np.asarray(vortex_feature, dtype=np.float32)
    pts_full = np.asarray(points, dtype=np.float32)

    y = vf[:, :, 0]
    x = vf[:, :, 1]
    tau = vf[:, :, 2]
    sig = vf[:, :, 3]
    sig2 = sig * sig
    nisg = -1.0 / sig2
    chalf = 0.5 * sig2

    def blk(a):  # [B, N] -> [128, B*NK] with col = b*NK+k
        return np.ascontiguousarray(
            a.reshape(B, NK, 128).transpose(2, 0, 1).reshape(128, B * NK)
        )

    params = np.zeros((128, 32), dtype=np.float32)
    params[:, 0:8] = blk(-y)
    params[:, 8:16] = blk(-x)
    params[:, 16:24] = blk(nisg)
    params[:, 24:32] = blk(chalf)

    wfull = np.stack([tau, tau * x, tau * y], axis=-1)  # [B, N, 3]
    wmat = np.ascontiguousarray(
        wfull.reshape(B, NK, 128, 3).transpose(2, 0, 1, 3).reshape(128, B * NK * 3)
    )

    in_maps = []
    for i in range(NCORES):
        sl = pts_full[:, i * HPC : (i + 1) * HPC]          # [B, 32, 256, 2]
        flat = sl.reshape(B, PPB, 2)
        pts = np.ascontiguousarray(flat.transpose(0, 2, 1))  # [B, 2, PPB]
        ptsf = np.ascontiguousarray(pts.reshape(B, 2, 128, PPB // 128))
        # pre-broadcast rows: [B, NT, 2, 128, 512]
        ptsb = np.ascontiguousarray(
            np.broadcast_to(
                pts.reshape(B, 2, NT, 1, 512).transpose(0, 2, 1, 3, 4),
                (B, NT, 2, 128, 512),
            )
        )
        in_maps.append({"params": params, "wmat": wmat, "ptsb": ptsb, "ptsf": ptsf})
    return in_maps


def _assemble(results):
    out = np.zeros((B, H, W, 2), dtype=np.float32)
    for i in range(NCORES):
        o = np.asarray(results[i]["out"])  # [B, 2, 128, PPB//128]
        o = o.reshape(B, 2, PPB).transpose(0, 2, 1).reshape(B, HPC, W, 2)
        out[:, i * HPC : (i + 1) * HPC] = o
    return out


def _run(vortex_feature, points, trace=False):
    _, _, _, run_bass_kernel_spmd, _b = _bass_modules()
    if "nc" not in _cache:
        _cache["nc"] = _build_nc()
    in_maps = _prep_inputs(vortex_feature, points)
    res = run_bass_kernel_spmd(
        _cache["nc"], in_maps, list(range(NCORES)), trace=trace
    )
    return _assemble(res.results), res


def kernel(vortex_feature, points):
    out, _ = _run(vortex_feature, points, trace=False)
    return out



# revision 2
# speedup vs baseline: 2.2697x; 2.2697x over previous
"""Gaussian falloff vortex-velocity kernel for Trainium2 (8 NeuronCores).

Math: out[b,h,w,:] = sum_n tau_n * exp(-r2/sig_n^2) / sqrt(r2) * (d2, -d1)
with d1 = py - y_n, d2 = px - x_n, r2 = d1^2 + d2^2.

Device algorithm (per core, H sharded 8 ways):
  1. PE computes t2' = a_n*(r2 + eps_n) for 128 particles x 512 points per
     matmul, where a_n = 2/sig_n^2, via a K=31 contraction of triple-bf16-split
     terms: a*py^2 - 2a*y*py + a*y^2 + a*px^2 - 2a*x*px + a*x^2 + a*eps.
     Rows are ordered so partial sums telescope near zero for close pairs,
     keeping fp32 accumulation error ~1e-6 in r2 units.
  2. ACT: lt = Ln(t2')                 (PSUM -> SBUF, fp32)
  3. DVE: w  = -t2' - lt               (one scalar_tensor_tensor, fp32)
  4. ACT: g  = Exp(0.5*w)              (-> bf16)  [= exp(-t2'/2)/sqrt(t2')]
  5. PE: S_r = sum_n w_rn * g_n  for r in {0,1,2} with hi/lo-split bf16
     weights {tau*q, tau*x*q, tau*y*q}, q = exp(a*eps/2)*sqrt(a).
  6. DVE: u = px*S0 - S1, v = S2 - py*S0 (after a DRAM relayout round-trip).
Ln and Exp share one ACT table set (natural_log_exp_and_others).
"""

import sys

import numpy as np

B, H, W, N = 2, 256, 256, 512
NCORES = 8
HPC = H // NCORES          # 32 rows per core
PPB = HPC * W              # 8192 points per batch per core
NT = PPB // 512            # 16 point-tiles of 512 per batch
NK = N // 128              # 4 particle blocks
KROWS = 31
EPS0, EPS1 = 2e-6, 1.5e-6

_cache = {}


def _bass_modules():
    if "/opt/trn_rl_repo" not in sys.path:
        sys.path.insert(0, "/opt/trn_rl_repo")
    import concourse.bass as bass
    import concourse.mybir as mybir
    import concourse.tile as tile
    from concourse import bacc
    from concourse.bass_utils import run_bass_kernel_spmd

    return bass, mybir, tile, run_bass_kernel_spmd, bacc


def _pin_act_table_set():
    """Make the table-load pass satisfy Ln/Exp only from the combined set so
    alternating Ln/Exp instructions never thrash ACT table loads."""
    import concourse.bacc as bacc_mod
    import concourse.mybir as mybir

    if getattr(bacc_mod, "_act_tables_pinned", False):
        return
    orig = bacc_mod.get_activation_tables
    ln_exp = {mybir.ActivationFunctionType.Ln, mybir.ActivationFunctionType.Exp}

    def patched(arch):
        tables = orig(arch)
        keep = "natural_log_exp_and_others"
        if keep not in tables:
            return tables
        return {
            name: (funcs if name == keep else (funcs - ln_exp))
            for name, funcs in tables.items()
        }

    bacc_mod.get_activation_tables = patched
    bacc_mod._act_tables_pinned = True


def _build_nc():
    bass, mybir, tile, _, bacc = _bass_modules()
    _pin_act_table_set()
    f32 = mybir.dt.float32
    bf16 = mybir.dt.bfloat16
    AF = mybir.ActivationFunctionType
    ALU = mybir.AluOpType

    nc = bacc.Bacc(None)
    rhs_d = nc.declare_dram_parameter("rhs", [B, KROWS, PPB], bf16, isOutput=False)
    lhst_d = nc.declare_dram_parameter("lhst", [B, KROWS, N], bf16, isOutput=False)
    wh_d = nc.declare_dram_parameter("wmath", [128, B * NK * 3], bf16, isOutput=False)
    wl_d = nc.declare_dram_parameter("wmatl", [128, B * NK * 3], bf16, isOutput=False)
    ptsf_d = nc.declare_dram_parameter("ptsf", [B, 2, 128, PPB // 128], f32, isOutput=False)
    out_d = nc.declare_dram_parameter("out", [B, 2, 128, PPB // 128], f32, isOutput=True)

    with tile.TileContext(nc) as tc:
        with (
            tc.tile_pool(name="const", bufs=1) as cpool,
            tc.tile_pool(name="lts", bufs=3) as ltpool,
            tc.tile_pool(name="wg", bufs=2) as wgpool,
            tc.tile_pool(name="stg", bufs=2) as stgpool,
            tc.tile_pool(name="fin", bufs=2) as fin,
            tc.tile_pool(name="r2p", bufs=3, space=bass.MemorySpace.PSUM) as r2pool,
            tc.tile_pool(name="sap", bufs=2, space=bass.MemorySpace.PSUM) as spool,
            tc.tile_pool(name="dscratch", bufs=1, space="DRAM") as dpool,
        ):
            rhs_sb, lhs_sb = [], []
            for b in range(B):
                t = cpool.tile([KROWS, PPB], bf16, tag=f"rhs{b}")
                nc.sync.dma_start(t[:], rhs_d[b])
                rhs_sb.append(t)
                t2 = cpool.tile([KROWS, N], bf16, tag=f"lhs{b}")
                nc.sync.dma_start(t2[:], lhst_d[b])
                lhs_sb.append(t2)
            wh = cpool.tile([128, B * NK * 3], bf16, tag="wh")
            nc.sync.dma_start(wh[:], wh_d[:])
            wl = cpool.tile([128, B * NK * 3], bf16, tag="wl")
            nc.sync.dma_start(wl[:], wl_d[:])
            scratch = dpool.tile([B, 3, PPB], f32)
            srs = scratch[:].rearrange("b three (p f) -> b three p f", p=128)

            for b in range(B):
                sstage = stgpool.tile([3, PPB], f32, tag="sstage")
                for T in range(NT):
                    sl = slice(T * 512, (T + 1) * 512)
                    wt = wgpool.tile([128, 2048], f32, tag="wt")
                    for p in range(2):
                        r2t = r2pool.tile([128, 1024], f32, tag="r2")
                        for hh in range(2):
                            k = 2 * p + hh
                            nc.tensor.matmul(
                                r2t[:, hh * 512 : (hh + 1) * 512],
                                lhs_sb[b][:, k * 128 : (k + 1) * 128],
                                rhs_sb[b][:, sl],
                                start=True,
                                stop=True,
                            )
                        lt = ltpool.tile([128, 1024], f32, tag="lt")
                        nc.scalar.activation(lt[:], r2t[:], AF.Ln)
                        nc.vector.scalar_tensor_tensor(
                            wt[:, p * 1024 : (p + 1) * 1024],
                            r2t[:],
                            -1.0,
                            lt[:],
                            ALU.mult,
                            ALU.subtract,
                        )
                    g = wgpool.tile([128, 2048], bf16, tag="g")
                    nc.scalar.activation(g[:], wt[:], AF.Exp, scale=0.5)
                    sacc = spool.tile([3, 512], f32, tag="sacc")
                    for k in range(NK):
                        c3 = (b * NK + k) * 3
                        gk = g[:, k * 512 : (k + 1) * 512]
                        nc.tensor.matmul(
                            sacc[:], wh[:, c3 : c3 + 3], gk,
                            start=(k == 0), stop=False,
                        )
                        nc.tensor.matmul(
                            sacc[:], wl[:, c3 : c3 + 3], gk,
                            start=False, stop=(k == NK - 1),
                        )
                    nc.vector.tensor_copy(sstage[:, sl], sacc[:])
                nc.sync.dma_start(scratch[b], sstage[:])

                s0 = fin.tile([128, PPB // 128], f32, tag="s0")
                nc.sync.dma_start(s0[:], srs[b, 0])
                s1 = fin.tile([128, PPB // 128], f32, tag="s1")
                nc.sync.dma_start(s1[:], srs[b, 1])
                s2 = fin.tile([128, PPB // 128], f32, tag="s2")
                nc.sync.dma_start(s2[:], srs[b, 2])
                pyf = fin.tile([128, PPB // 128], f32, tag="pyf")
                nc.sync.dma_start(pyf[:], ptsf_d[b, 0])
                pxf = fin.tile([128, PPB // 128], f32, tag="pxf")
                nc.sync.dma_start(pxf[:], ptsf_d[b, 1])
                tu = fin.tile([128, PPB // 128], f32, tag="tu")
                nc.vector.tensor_mul(tu[:], pxf[:], s0[:])
                u = fin.tile([128, PPB // 128], f32, tag="u")
                nc.vector.tensor_sub(u[:], tu[:], s1[:])
                tv = fin.tile([128, PPB // 128], f32, tag="tv")
                nc.vector.tensor_mul(tv[:], pyf[:], s0[:])
                v = fin.tile([128, PPB // 128], f32, tag="v")
                nc.vector.tensor_sub(v[:], s2[:], tv[:])
                nc.sync.dma_start(out_d[b, 0], u[:])
                nc.sync.dma_start(out_d[b, 1], v[:])
    nc.compile()
    return nc


def _split3(a, bf):
    h = a.astype(bf)
    m = (a - h.astype(np.float64)).astype(bf)
    l = (a - h.astype(np.float64) - m.astype(np.float64)).astype(bf)
    return h, m, l


def _prep_inputs(vortex_feature, points):
    import ml_dtypes

    bf = ml_dtypes.bfloat16
    vf = np.asarray(vortex_feature, dtype=np.float64)
    pts_full = np.asarray(points, dtype=np.float64)
    y, x, tau = vf[:, :, 0], vf[:, :, 1], vf[:, :, 2]
    sig2 = vf[:, :, 3] ** 2
    a_n = 2.0 / sig2
    eps_n = EPS0 + EPS1 * (y * y + x * x)

    # lhsT rows [B, KROWS, N]: triple-split entries; order must match rhs rows.
    lhst = np.zeros((B, KROWS, N), dtype=bf)
    for b in range(B):
        A3 = _split3(a_n[b], bf)
        CY3 = _split3(-2.0 * a_n[b] * y[b], bf)
        CX3 = _split3(-2.0 * a_n[b] * x[b], bf)
        AYY3 = _split3(a_n[b] * y[b] * y[b], bf)
        AXX3 = _split3(a_n[b] * x[b] * x[b], bf)
        aeps = (a_n[b] * eps_n[b]).astype(bf)
        rows = []
        for (uh, um, ul) in (A3, CY3):
            rows += [uh, uh, um, uh, ul, um]
        rows += list(AYY3)
        for (uh, um, ul) in (A3, CX3):
            rows += [uh, uh, um, uh, ul, um]
        rows += list(AXX3)
        rows.append(aeps)
        lhst[b] = np.stack(rows, 0)

    # weights with eps correction, hi/lo split, [128, B*NK*3]
    q = np.exp(0.5 * a_n * eps_n) * np.sqrt(a_n)
    wfull = np.stack([tau * q, tau * x * q, tau * y * q], axis=-1)  # [B, N, 3]
    whd = wfull.astype(bf)
    wld = (wfull - whd.astype(np.float64)).astype(bf)

    def wlay(a):  # [B, N, 3] -> [128, B*NK*3]
        return np.ascontiguousarray(
            a.reshape(B, NK, 128, 3).transpose(2, 0, 1, 3).reshape(128, B * NK * 3)
        )

    whm, wlm = wlay(whd), wlay(wld)

    in_maps = []
    for i in range(NCORES):
        slp = pts_full[:, i * HPC : (i + 1) * HPC].reshape(B, PPB, 2)
        pts = np.ascontiguousarray(slp.transpose(0, 2, 1))  # [B, 2, PPB]
        ptsf = np.ascontiguousarray(
            pts.reshape(B, 2, 128, PPB // 128), dtype=np.float32
        )
        rhs = np.zeros((B, KROWS, PPB), dtype=bf)
        for b in range(B):
            py, px = pts[b, 0], pts[b, 1]
            PYY3 = _split3(py * py, bf)
            PY3 = _split3(py, bf)
            PXX3 = _split3(px * px, bf)
            PX3 = _split3(px, bf)
            ones = np.ones(PPB, dtype=bf)
            rows = []
            for (wh_, wm_, wl_) in (PYY3, PY3):
                rows += [wh_, wm_, wh_, wl_, wh_, wm_]
            rows += [ones] * 3
            for (wh_, wm_, wl_) in (PXX3, PX3):
                rows += [wh_, wm_, wh_, wl_, wh_, wm_]
            rows += [ones] * 3
            rows.append(ones)
            rhs[b] = np.stack(rows, 0)
        in_maps.append(
            {"rhs": rhs, "lhst": lhst, "wmath": whm, "wmatl": wlm, "ptsf": ptsf}
        )
    return in_maps


def _assemble(results):
    out = np.zeros((B, H, W, 2), dtype=np.float32)
    for i in range(NCORES):
        o = np.asarray(results[i]["out"])  # [B, 2, 128, PPB//128]
        o = o.reshape(B, 2, PPB).transpose(0, 2, 1).reshape(B, HPC, W, 2)
        out[:, i * HPC : (i + 1) * HPC] = o
    return out


def _run(vortex_feature, points, trace=False):
    _, _, _, run_bass_kernel_spmd, _b = _bass_modules()
    if "nc" not in _cache:
        _cache["nc"] = _build_nc()
    in_maps = _prep_inputs(vortex_feature, points)
    res = run_bass_kernel_spmd(
        _cache["nc"], in_maps, list(range(NCORES)), trace=trace
    )
    return _assemble(res.results), res


def kernel(vortex_feature, points):
    out, _ = _run(vortex_feature, points, trace=False)
    return out
